# revision 36
# baseline (speedup 1.0000x reference)
"""Trainium2 Bass kernel for nn_MlpMixer_18966575579742 (bf16 rewrite).

Complex-valued per-frequency (j) MLP:
  o1r = gelu(xr@w1[0] - xi@w1[1] + b1[0]);  o1i = gelu(xi@w1[0] + xr@w1[1] + b1[1])
  o2r = o1r@w2[0] - o1i@w2[1] + b2[0];      o2i = o1i@w2[0] + o1i@w2[1] + b2[1]
  (note: o2i intentionally uses o1i with BOTH w2[0] and w2[1], as in the source)

Sharding over 8 cores: 2 j-halves (13 each) x 4 batch-quarters (B=32 -> 512 rows).

Key differences vs the fp32 baseline (289us):
  - ALL matmul operands are bf16 (fp32 matmul = 2 HW passes + 2x DMA bytes;
    tolerance is 2e-2 absmax so bf16's ~5e-3 is plenty). 312 passes/core.
  - L1 is the DIRECT 4-matmul complex product accumulated in PSUM
    (pre_r = w1[0]^T@xr + (-w1[1])^T@xi etc.), so the Gauss-trick's 156 DVE
    combine ops are gone entirely; GELU+bias reads PSUM directly on ScalarE.
  - L2 uses the algebraic identity o2r + o2i_pre = (o1r+o1i)@w2[0] [since
    o2i_pre = o1i@(w2[0]+w2[1])]: only 2 matmuls per h-chunk:
      T = o1i@(w2[0]+w2[1])   (= o2i pre-bias)
      S = (o1r+o1i)@w2[0]     (o2r = S - T + b2r)
    costing one bf16 DVE add per h-chunk (o1s = o1r + o1i).
  - per-j DMA is 3 big contiguous transfers (x-pair 2KB/part, weight-pack
    5KB/part incl. host-negated -w1[1] and host-summed w2[0]+w2[1], out-pair
    2KB/part); weight DMAs issue on gpsimd, x/out on sync.
  - L2 matmuls are emitted with a 2-slot lag behind L1 (pending deque) so the
    PE never waits on the ScalarE GELU -> DVE add chain.
  - PSUM: 4 banks rotate L1 pre-tiles (2 h-chunks in flight), 4 banks rotate
    L2 T/S accumulators (2 j in flight). Exactly 8.
  - a dummy GELU at kernel start pulls the ~2.7us ACT table load under the
    initial DMA wait.
"""

import sys

if "/opt/trn_rl_repo" not in sys.path:
    sys.path.insert(0, "/opt/trn_rl_repo")

from collections import deque

import numpy as np
from ml_dtypes import bfloat16

B, I, J, K, F = 128, 16, 26, 128, 4
H = K * F  # 512
NJG = 2  # j groups
NRG = 4  # row (batch) groups
JL = J // NJG  # 13 j per core
BL = B // NRG  # 32 batches per core
ROWS = BL * I  # 512 rows per core
NHC = H // 128  # 4 h-chunks
WCOLS = 4 * H  # w1[0] | w1[1] | w2[0] | w2[0]+w2[1]  (-w1[1] negated on DVE)

_cache = {}


def _build_nc():
    from contextlib import ExitStack

    import concourse.mybir as mybir
    import concourse.tile as tile
    from concourse import bacc

    f32 = mybir.dt.float32
    bf16 = mybir.dt.bfloat16
    nc = bacc.Bacc(None)

    # x pre-transposed on host: [j, k, rows*2] = [xr | xi]
    xp = nc.declare_dram_parameter("xp", [JL, K, 2 * ROWS], bf16, isOutput=False)
    # weight pack: [j, 128, 5*H]; first 3 slots partition=k, last 2 partition=h%128
    wp = nc.declare_dram_parameter("wp", [JL, 128, WCOLS], bf16, isOutput=False)
    # biases host-pre-transposed to per-partition layout
    b1d = nc.declare_dram_parameter("b1t", [128, 2 * JL * NHC], f32, isOutput=False)
    b2d = nc.declare_dram_parameter("b2t", [128, 2 * JL], f32, isOutput=False)
    # transposed output: [j, k', rows*2] = [real | imag]; host fixes layout
    out = nc.declare_dram_parameter("out", [JL, K, 2 * ROWS], bf16, isOutput=True)

    GELU = mybir.ActivationFunctionType.Gelu

    with tile.TileContext(nc) as tc, ExitStack() as ctx:
        const = ctx.enter_context(tc.tile_pool(name="const", bufs=1))
        xpool = ctx.enter_context(tc.tile_pool(name="xpool", bufs=4))
        wpool = ctx.enter_context(tc.tile_pool(name="wpool", bufs=4))
        w1np = ctx.enter_context(tc.tile_pool(name="w1np", bufs=2))
        o1p = ctx.enter_context(tc.tile_pool(name="o1p", bufs=2))
        srpp = ctx.enter_context(tc.tile_pool(name="srpp", bufs=2))
        outp = ctx.enter_context(tc.tile_pool(name="outp", bufs=3))
        ps1 = ctx.enter_context(tc.tile_pool(name="ps1", bufs=4, space="PSUM"))
        ps2 = ctx.enter_context(tc.tile_pool(name="ps2", bufs=4, space="PSUM"))

        # warm tile for HAM warm-up matmuls + gelu table preload; memset on
        # gpsimd whose preamble finishes earliest, so warm-up starts ASAP
        cwarm = const.tile([128, ROWS], bf16)
        nc.gpsimd.memset(cwarm, 0)
        warm = const.tile([128, 1], bf16)
        nc.scalar.activation(warm, cwarm[:, 0:1], GELU)

        # biases arrive already transposed; straight DMA on the idle gpsimd
        # queue, issued first so they land before the first GELU
        b1t = const.tile([128, 2, JL, NHC], f32)
        nc.gpsimd.dma_start(out=b1t.rearrange("p c j hc -> p (c j hc)"), in_=b1d[:, :])
        b2t = const.tile([128, 2, JL], f32)
        nc.gpsimd.dma_start(out=b2t.rearrange("p c j -> p (c j)"), in_=b2d[:, :])

        jstate = {}

        def start_j(j, sliced=False):
            wt = wpool.tile([128, WCOLS], bf16, tag="wt")
            xt = xpool.tile([128, 2 * ROWS], bf16, tag="xt")
            if sliced:
                # all 8 cores hammer HBM simultaneously at kernel start, so a
                # monolithic first transfer gates the first matmul ~10us out.
                # Slice j0's data in compute-consumption order instead.
                nc.sync.dma_start(out=wt[:, 0:H], in_=wp[j, :, 0:H])  # w1[0]
                nc.sync.dma_start(out=xt[:, 0:ROWS], in_=xp[j, :, 0:ROWS])  # xr
                nc.sync.dma_start(out=wt[:, H : 2 * H], in_=wp[j, :, H : 2 * H])
                nc.sync.dma_start(out=xt[:, ROWS:], in_=xp[j, :, ROWS:])  # xi
                nc.sync.dma_start(out=wt[:, 2 * H :], in_=wp[j, :, 2 * H :])  # w2
            else:
                nc.sync.dma_start(out=wt, in_=wp[j])
                nc.sync.dma_start(out=xt, in_=xp[j])
            jstate[j] = (xt, wt)

        def negate_w1(j):
            # w1n = -w1[1] for the pre_r accumulation (bf16 DVE, ~0.2us)
            xt, wt = jstate[j]
            w1n = w1np.tile([128, H], bf16, tag="w1n")
            nc.vector.tensor_scalar_mul(w1n, wt[:, H : 2 * H], -1.0)
            jstate[j] = (xt, wt, w1n)

        start_j(0, sliced=True)
        start_j(1)
        negate_w1(0)

        # dummy matmuls while the first DMAs land: flips the PE HAM clock
        # gate to 8/8 so the first real passes run at 2.4 GHz
        NWARM = 0
        warmps = ps2.tile([128, ROWS], f32, tag="ps2")
        for wi in range(NWARM):
            nc.tensor.matmul(
                warmps, cwarm[:, 0:128], cwarm,
                start=(wi == 0), stop=(wi == NWARM - 1), skip_group_check=True,
            )

        TS = {}  # j -> (T, S) psum accumulators, allocated at first L2 pop

        def emit_L2(j, hc, wt, o1i, o1s):
            if hc == 0:
                T = ps2.tile([128, ROWS], f32, tag="ps2")
                S = ps2.tile([128, ROWS], f32, tag="ps2")
                TS[j] = (T, S)
            T, S = TS[j]
            c0 = 3 * H + hc * 128  # w2sum slot
            nc.tensor.matmul(
                T, wt[:, c0 : c0 + 128], o1i[:, hc],
                start=(hc == 0), stop=(hc == NHC - 1),
            )
            c1 = 2 * H + hc * 128  # w2[0] slot
            nc.tensor.matmul(
                S, wt[:, c1 : c1 + 128], o1s[:, hc],
                start=(hc == 0), stop=(hc == NHC - 1),
            )

        def emit_drain(j):
            T, S = TS.pop(j)
            ot = outp.tile([128, 2 * ROWS], bf16, tag="ot")
            # imag: T + b2i
            nc.vector.tensor_scalar_add(ot[:, ROWS:], T, b2t[:, 1, j : j + 1])
            last = j == JL - 1
            if last:
                # ship the imag half immediately so the store overlaps the
                # remaining two DVE ops on the critical tail
                nc.sync.dma_start(out=out[j, :, ROWS:], in_=ot[:, ROWS:])
            # real: (S + b2r) - T
            srp = srpp.tile([128, ROWS], f32, tag="srp")
            nc.vector.tensor_scalar_add(srp, S, b2t[:, 0, j : j + 1])
            nc.vector.tensor_sub(ot[:, 0:ROWS], srp, T)
            if last:
                nc.sync.dma_start(out=out[j, :, 0:ROWS], in_=ot[:, 0:ROWS])
            else:
                nc.sync.dma_start(out=out[j], in_=ot)

        pend = deque()
        for j in range(JL):
            # staged prefetch ramp: depth 2 during the startup HBM crunch
            # (all 8 cores pull at once), depth 3 in steady state
            if j == 0:
                start_j(2)
            elif j + 3 < JL:
                start_j(j + 3)
            xt, wt, w1n = jstate.pop(j)
            xr_ = xt[:, 0:ROWS]
            xi_ = xt[:, ROWS:]
            o1r = o1p.tile([128, NHC, ROWS], bf16, tag="o1r")
            o1i = o1p.tile([128, NHC, ROWS], bf16, tag="o1i")
            o1s = o1p.tile([128, NHC, ROWS], bf16, tag="o1s")
            for hc in range(NHC):
                hb = hc * 128
                pr = ps1.tile([128, ROWS], f32, tag="ps1")
                pi = ps1.tile([128, ROWS], f32, tag="ps1")
                # pre_r = w1[0]^T @ xr + (-w1[1])^T @ xi   (finished first so
                # the o1r GELU can start while the pre_i passes still run)
                # pre_i = w1[0]^T @ xi +   w1[1]^T @ xr
                nc.tensor.matmul(
                    pr, wt[:, hb : hb + 128], xr_, start=True, stop=False
                )
                nc.tensor.matmul(
                    pr, w1n[:, hb : hb + 128], xi_, start=False, stop=True
                )
                # the lagged L2 passes of older h-chunks sit between pre_r
                # and pre_i, so pre_r closes early relative to the GELU that
                # consumes it (drop the lag on the last j to shorten the tail)
                while len(pend) > (1 if j < JL - 1 else 0):
                    pj, phc, pwt, po1i, po1s = pend.popleft()
                    emit_L2(pj, phc, pwt, po1i, po1s)
                    if phc == NHC - 1:
                        emit_drain(pj)
                nc.tensor.matmul(
                    pi, wt[:, hb : hb + 128], xi_, start=True, stop=False
                )
                nc.tensor.matmul(
                    pi, wt[:, H + hb : H + hb + 128], xr_, start=False, stop=True
                )
                nc.scalar.activation(
                    o1r[:, hc], pr, GELU, bias=b1t[:, 0, j, hc : hc + 1]
                )
                nc.scalar.activation(
                    o1i[:, hc], pi, GELU, bias=b1t[:, 1, j, hc : hc + 1]
                )
                nc.vector.tensor_add(o1s[:, hc], o1r[:, hc], o1i[:, hc])
                pend.append((j, hc, wt, o1i, o1s))
            if j == 0:
                start_j(3)
            if j + 1 < JL:
                # -w1[1] for the next j; its wt DMA landed during this body,
                # so this never blocks the DVE queue head
                negate_w1(j + 1)
        while pend:
            pj, phc, pwt, po1i, po1s = pend.popleft()
            emit_L2(pj, phc, pwt, po1i, po1s)
            if phc == NHC - 1:
                emit_drain(pj)

    if not nc.is_finalized():
        nc.finalize()
    return nc


def _prep_shards(x_real, x_imag, w1, b1, w2, b2):
    """Host-side packing. Returns one input map per core (8 = 2 jg x 4 rg)."""
    wpks, b1l, b2l = [], [], []
    for jg in range(NJG):
        js = slice(jg * JL, (jg + 1) * JL)
        w10 = w1[0, js]  # [JL, K, H] partition=k
        w11 = w1[1, js]
        w2z = w2[0, js]  # [JL, H, K]
        w2sum = w2[0, js] + w2[1, js]
        # [JL, H, K] -> [JL, 128, NHC*K] with partition = h % 128
        w2z_r = (
            w2z.reshape(JL, NHC, 128, K).transpose(0, 2, 1, 3).reshape(JL, 128, NHC * K)
        )
        w2s_r = (
            w2sum.reshape(JL, NHC, 128, K)
            .transpose(0, 2, 1, 3)
            .reshape(JL, 128, NHC * K)
        )
        wpk = np.concatenate([w10, w11, w2z_r, w2s_r], axis=2).astype(bfloat16)
        wpks.append(np.ascontiguousarray(wpk))
        # pre-transpose biases to the on-chip per-partition layout
        b1t = (
            b1[:, js]
            .reshape(2, JL, NHC, 128)
            .transpose(3, 0, 1, 2)
            .reshape(128, 2 * JL * NHC)
        )
        b2t = b2[:, js].transpose(2, 0, 1).reshape(128, 2 * JL)
        b1l.append(np.ascontiguousarray(b1t))
        b2l.append(np.ascontiguousarray(b2t))

    in_maps = []
    for jg in range(NJG):
        js = slice(jg * JL, (jg + 1) * JL)
        for rg in range(NRG):
            bs = slice(rg * BL, (rg + 1) * BL)
            # [BL, I, JL, K] -> [JL, K, BL*I]
            xr_s = x_real[bs, :, js, :].transpose(2, 3, 0, 1).reshape(JL, K, ROWS)
            xi_s = x_imag[bs, :, js, :].transpose(2, 3, 0, 1).reshape(JL, K, ROWS)
            xpk = np.concatenate([xr_s, xi_s], axis=2).astype(bfloat16)
            in_maps.append(
                {
                    "xp": np.ascontiguousarray(xpk),
                    "wp": wpks[jg],
                    "b1t": b1l[jg],
                    "b2t": b2l[jg],
                }
            )
    return in_maps


def _gather(results):
    out = np.empty((B, I, J, K), np.complex64)
    idx = 0
    for jg in range(NJG):
        for rg in range(NRG):
            js = slice(jg * JL, (jg + 1) * JL)
            bs = slice(rg * BL, (rg + 1) * BL)
            o = np.asarray(results[idx]["out"]).astype(np.float32)  # [13,128,1024]
            oc = (o[:, :, :ROWS] + 1j * o[:, :, ROWS:]).astype(np.complex64)
            # [j, k, rows] -> [rows, j, k] -> [BL, I, JL, K]
            out[bs, :, js, :] = oc.transpose(2, 0, 1).reshape(BL, I, JL, K)
            idx += 1
    return out


def run(trace=False, **inputs):
    from concourse.bass_utils import run_bass_kernel_spmd

    if "nc" not in _cache:
        _cache["nc"] = _build_nc()
    in_maps = _prep_shards(
        np.asarray(inputs["x_real"], np.float32),
        np.asarray(inputs["x_imag"], np.float32),
        np.asarray(inputs["w1"], np.float32),
        np.asarray(inputs["b1"], np.float32),
        np.asarray(inputs["w2"], np.float32),
        np.asarray(inputs["b2"], np.float32),
    )
    res = run_bass_kernel_spmd(_cache["nc"], in_maps, list(range(8)), trace=trace)
    return _gather(res.results), res


def kernel(**inputs):
    out, _ = run(trace=False, **inputs)
    return out


# revision 37
# speedup vs baseline: 1.0117x; 1.0117x over previous
"""Trainium2 Bass kernel for nn_MlpMixer_18966575579742 (bf16 rewrite).

Complex-valued per-frequency (j) MLP:
  o1r = gelu(xr@w1[0] - xi@w1[1] + b1[0]);  o1i = gelu(xi@w1[0] + xr@w1[1] + b1[1])
  o2r = o1r@w2[0] - o1i@w2[1] + b2[0];      o2i = o1i@w2[0] + o1i@w2[1] + b2[1]
  (note: o2i intentionally uses o1i with BOTH w2[0] and w2[1], as in the source)

Sharding over 8 cores: 2 j-halves (13 each) x 4 batch-quarters (B=32 -> 512 rows).

Key differences vs the fp32 baseline (289us):
  - ALL matmul operands are bf16 (fp32 matmul = 2 HW passes + 2x DMA bytes;
    tolerance is 2e-2 absmax so bf16's ~5e-3 is plenty). 312 passes/core.
  - L1 is the DIRECT 4-matmul complex product accumulated in PSUM
    (pre_r = w1[0]^T@xr + (-w1[1])^T@xi etc.), so the Gauss-trick's 156 DVE
    combine ops are gone entirely; GELU+bias reads PSUM directly on ScalarE.
  - L2 uses the algebraic identity o2r + o2i_pre = (o1r+o1i)@w2[0] [since
    o2i_pre = o1i@(w2[0]+w2[1])]: only 2 matmuls per h-chunk:
      T = o1i@(w2[0]+w2[1])   (= o2i pre-bias)
      S = (o1r+o1i)@w2[0]     (o2r = S - T + b2r)
    costing one bf16 DVE add per h-chunk (o1s = o1r + o1i).
  - per-j DMA is 3 big contiguous transfers (x-pair 2KB/part, weight-pack
    5KB/part incl. host-negated -w1[1] and host-summed w2[0]+w2[1], out-pair
    2KB/part); weight DMAs issue on gpsimd, x/out on sync.
  - L2 matmuls are emitted with a 2-slot lag behind L1 (pending deque) so the
    PE never waits on the ScalarE GELU -> DVE add chain.
  - PSUM: 4 banks rotate L1 pre-tiles (2 h-chunks in flight), 4 banks rotate
    L2 T/S accumulators (2 j in flight). Exactly 8.
  - a dummy GELU at kernel start pulls the ~2.7us ACT table load under the
    initial DMA wait.
"""

import sys

if "/opt/trn_rl_repo" not in sys.path:
    sys.path.insert(0, "/opt/trn_rl_repo")

from collections import deque

import numpy as np
from ml_dtypes import bfloat16

B, I, J, K, F = 128, 16, 26, 128, 4
H = K * F  # 512
NJG = 2  # j groups
NRG = 4  # row (batch) groups
JL = J // NJG  # 13 j per core
BL = B // NRG  # 32 batches per core
ROWS = BL * I  # 512 rows per core
NHC = H // 128  # 4 h-chunks
WCOLS = 4 * H  # w1[0] | w1[1] | w2[0] | w2[0]+w2[1]  (-w1[1] negated on DVE)

_cache = {}


def _build_nc():
    from contextlib import ExitStack

    import concourse.mybir as mybir
    import concourse.tile as tile
    from concourse import bacc

    f32 = mybir.dt.float32
    bf16 = mybir.dt.bfloat16
    nc = bacc.Bacc(None)

    # x pre-transposed on host: [j, k, rows*2] = [xr | xi]
    xp = nc.declare_dram_parameter("xp", [JL, K, 2 * ROWS], bf16, isOutput=False)
    # weight pack: [j, 128, 5*H]; first 3 slots partition=k, last 2 partition=h%128
    wp = nc.declare_dram_parameter("wp", [JL, 128, WCOLS], bf16, isOutput=False)
    # biases host-pre-transposed to per-partition layout
    b1d = nc.declare_dram_parameter("b1t", [128, 2 * JL * NHC], f32, isOutput=False)
    b2d = nc.declare_dram_parameter("b2t", [128, 2 * JL], f32, isOutput=False)
    # transposed output: [j, k', rows*2] = [real | imag]; host fixes layout
    out = nc.declare_dram_parameter("out", [JL, K, 2 * ROWS], bf16, isOutput=True)

    GELU = mybir.ActivationFunctionType.Gelu

    with tile.TileContext(nc) as tc, ExitStack() as ctx:
        const = ctx.enter_context(tc.tile_pool(name="const", bufs=1))
        xpool = ctx.enter_context(tc.tile_pool(name="xpool", bufs=4))
        wpool = ctx.enter_context(tc.tile_pool(name="wpool", bufs=4))
        w1np = ctx.enter_context(tc.tile_pool(name="w1np", bufs=2))
        o1p = ctx.enter_context(tc.tile_pool(name="o1p", bufs=2))
        srpp = ctx.enter_context(tc.tile_pool(name="srpp", bufs=2))
        outp = ctx.enter_context(tc.tile_pool(name="outp", bufs=3))
        ps1 = ctx.enter_context(tc.tile_pool(name="ps1", bufs=4, space="PSUM"))
        ps2 = ctx.enter_context(tc.tile_pool(name="ps2", bufs=4, space="PSUM"))

        # warm tile for HAM warm-up matmuls + gelu table preload; memset on
        # gpsimd whose preamble finishes earliest, so warm-up starts ASAP
        cwarm = const.tile([128, ROWS], bf16)
        nc.gpsimd.memset(cwarm, 0)
        warm = const.tile([128, 1], bf16)
        nc.scalar.activation(warm, cwarm[:, 0:1], GELU)

        # biases arrive already transposed; straight DMA on the idle gpsimd
        # queue, issued first so they land before the first GELU
        b1t = const.tile([128, 2, JL, NHC], f32)
        nc.gpsimd.dma_start(out=b1t.rearrange("p c j hc -> p (c j hc)"), in_=b1d[:, :])
        b2t = const.tile([128, 2, JL], f32)
        nc.gpsimd.dma_start(out=b2t.rearrange("p c j -> p (c j)"), in_=b2d[:, :])

        jstate = {}

        def start_j(j, sliced=False):
            wt = wpool.tile([128, WCOLS], bf16, tag="wt")
            xt = xpool.tile([128, 2 * ROWS], bf16, tag="xt")
            if sliced:
                # all 8 cores hammer HBM simultaneously at kernel start, so a
                # monolithic first transfer gates the first matmul ~10us out.
                # Slice j0's data in compute-consumption order instead.
                nc.sync.dma_start(out=wt[:, 0:H], in_=wp[j, :, 0:H])  # w1[0]
                nc.sync.dma_start(out=xt[:, 0:ROWS], in_=xp[j, :, 0:ROWS])  # xr
                nc.sync.dma_start(out=wt[:, H : 2 * H], in_=wp[j, :, H : 2 * H])
                nc.sync.dma_start(out=xt[:, ROWS:], in_=xp[j, :, ROWS:])  # xi
                nc.sync.dma_start(out=wt[:, 2 * H :], in_=wp[j, :, 2 * H :])  # w2
            else:
                nc.sync.dma_start(out=wt, in_=wp[j])
                nc.sync.dma_start(out=xt, in_=xp[j])
            jstate[j] = (xt, wt)

        def negate_w1(j):
            # w1n = -w1[1] for the pre_r accumulation (bf16 DVE, ~0.2us)
            xt, wt = jstate[j]
            w1n = w1np.tile([128, H], bf16, tag="w1n")
            nc.vector.tensor_scalar_mul(w1n, wt[:, H : 2 * H], -1.0)
            jstate[j] = (xt, wt, w1n)

        start_j(0, sliced=True)
        start_j(1)
        negate_w1(0)

        # dummy matmuls while the first DMAs land: flips the PE HAM clock
        # gate to 8/8 so the first real passes run at 2.4 GHz
        NWARM = 9
        warmps = ps2.tile([128, ROWS], f32, tag="ps2")
        for wi in range(NWARM):
            nc.tensor.matmul(
                warmps, cwarm[:, 0:128], cwarm,
                start=(wi == 0), stop=(wi == NWARM - 1), skip_group_check=True,
            )

        TS = {}  # j -> (T, S) psum accumulators, allocated at first L2 pop

        def emit_L2(j, hc, wt, o1i, o1s):
            if hc == 0:
                T = ps2.tile([128, ROWS], f32, tag="ps2")
                S = ps2.tile([128, ROWS], f32, tag="ps2")
                TS[j] = (T, S)
            T, S = TS[j]
            c0 = 3 * H + hc * 128  # w2sum slot
            nc.tensor.matmul(
                T, wt[:, c0 : c0 + 128], o1i[:, hc],
                start=(hc == 0), stop=(hc == NHC - 1),
            )
            c1 = 2 * H + hc * 128  # w2[0] slot
            nc.tensor.matmul(
                S, wt[:, c1 : c1 + 128], o1s[:, hc],
                start=(hc == 0), stop=(hc == NHC - 1),
            )

        def emit_drain(j):
            T, S = TS.pop(j)
            ot = outp.tile([128, 2 * ROWS], bf16, tag="ot")
            # imag: T + b2i
            nc.vector.tensor_scalar_add(ot[:, ROWS:], T, b2t[:, 1, j : j + 1])
            last = j == JL - 1
            if last:
                # ship the imag half immediately so the store overlaps the
                # remaining two DVE ops on the critical tail
                nc.sync.dma_start(out=out[j, :, ROWS:], in_=ot[:, ROWS:])
            # real: (S + b2r) - T
            srp = srpp.tile([128, ROWS], f32, tag="srp")
            nc.vector.tensor_scalar_add(srp, S, b2t[:, 0, j : j + 1])
            nc.vector.tensor_sub(ot[:, 0:ROWS], srp, T)
            if last:
                nc.sync.dma_start(out=out[j, :, 0:ROWS], in_=ot[:, 0:ROWS])
            else:
                nc.sync.dma_start(out=out[j], in_=ot)

        pend = deque()
        for j in range(JL):
            # staged prefetch ramp: depth 2 during the startup HBM crunch
            # (all 8 cores pull at once), depth 3 in steady state
            if j == 0:
                start_j(2)
            elif j + 3 < JL:
                start_j(j + 3)
            xt, wt, w1n = jstate.pop(j)
            xr_ = xt[:, 0:ROWS]
            xi_ = xt[:, ROWS:]
            o1r = o1p.tile([128, NHC, ROWS], bf16, tag="o1r")
            o1i = o1p.tile([128, NHC, ROWS], bf16, tag="o1i")
            o1s = o1p.tile([128, NHC, ROWS], bf16, tag="o1s")
            for hc in range(NHC):
                hb = hc * 128
                pr = ps1.tile([128, ROWS], f32, tag="ps1")
                pi = ps1.tile([128, ROWS], f32, tag="ps1")
                # pre_r = w1[0]^T @ xr + (-w1[1])^T @ xi   (finished first so
                # the o1r GELU can start while the pre_i passes still run)
                # pre_i = w1[0]^T @ xi +   w1[1]^T @ xr
                nc.tensor.matmul(
                    pr, wt[:, hb : hb + 128], xr_, start=True, stop=False
                )
                nc.tensor.matmul(
                    pr, w1n[:, hb : hb + 128], xi_, start=False, stop=True
                )
                # the lagged L2 passes of older h-chunks sit between pre_r
                # and pre_i, so pre_r closes early relative to the GELU that
                # consumes it (drop the lag on the last j to shorten the tail)
                while len(pend) > (1 if j < JL - 1 else 0):
                    pj, phc, pwt, po1i, po1s = pend.popleft()
                    emit_L2(pj, phc, pwt, po1i, po1s)
                    if phc == NHC - 1:
                        emit_drain(pj)
                nc.tensor.matmul(
                    pi, wt[:, hb : hb + 128], xi_, start=True, stop=False
                )
                nc.tensor.matmul(
                    pi, wt[:, H + hb : H + hb + 128], xr_, start=False, stop=True
                )
                nc.scalar.activation(
                    o1r[:, hc], pr, GELU, bias=b1t[:, 0, j, hc : hc + 1]
                )
                nc.scalar.activation(
                    o1i[:, hc], pi, GELU, bias=b1t[:, 1, j, hc : hc + 1]
                )
                nc.vector.tensor_add(o1s[:, hc], o1r[:, hc], o1i[:, hc])
                pend.append((j, hc, wt, o1i, o1s))
            if j == 0:
                start_j(3)
            if j + 1 < JL:
                # -w1[1] for the next j; its wt DMA landed during this body,
                # so this never blocks the DVE queue head
                negate_w1(j + 1)
        while pend:
            pj, phc, pwt, po1i, po1s = pend.popleft()
            emit_L2(pj, phc, pwt, po1i, po1s)
            if phc == NHC - 1:
                emit_drain(pj)

    if not nc.is_finalized():
        nc.finalize()
    return nc


def _prep_shards(x_real, x_imag, w1, b1, w2, b2):
    """Host-side packing. Returns one input map per core (8 = 2 jg x 4 rg)."""
    wpks, b1l, b2l = [], [], []
    for jg in range(NJG):
        js = slice(jg * JL, (jg + 1) * JL)
        w10 = w1[0, js]  # [JL, K, H] partition=k
        w11 = w1[1, js]
        w2z = w2[0, js]  # [JL, H, K]
        w2sum = w2[0, js] + w2[1, js]
        # [JL, H, K] -> [JL, 128, NHC*K] with partition = h % 128
        w2z_r = (
            w2z.reshape(JL, NHC, 128, K).transpose(0, 2, 1, 3).reshape(JL, 128, NHC * K)
        )
        w2s_r = (
            w2sum.reshape(JL, NHC, 128, K)
            .transpose(0, 2, 1, 3)
            .reshape(JL, 128, NHC * K)
        )
        wpk = np.concatenate([w10, w11, w2z_r, w2s_r], axis=2).astype(bfloat16)
        wpks.append(np.ascontiguousarray(wpk))
        # pre-transpose biases to the on-chip per-partition layout
        b1t = (
            b1[:, js]
            .reshape(2, JL, NHC, 128)
            .transpose(3, 0, 1, 2)
            .reshape(128, 2 * JL * NHC)
        )
        b2t = b2[:, js].transpose(2, 0, 1).reshape(128, 2 * JL)
        b1l.append(np.ascontiguousarray(b1t))
        b2l.append(np.ascontiguousarray(b2t))

    in_maps = []
    for jg in range(NJG):
        js = slice(jg * JL, (jg + 1) * JL)
        for rg in range(NRG):
            bs = slice(rg * BL, (rg + 1) * BL)
            # [BL, I, JL, K] -> [JL, K, BL*I]
            xr_s = x_real[bs, :, js, :].transpose(2, 3, 0, 1).reshape(JL, K, ROWS)
            xi_s = x_imag[bs, :, js, :].transpose(2, 3, 0, 1).reshape(JL, K, ROWS)
            xpk = np.concatenate([xr_s, xi_s], axis=2).astype(bfloat16)
            in_maps.append(
                {
                    "xp": np.ascontiguousarray(xpk),
                    "wp": wpks[jg],
                    "b1t": b1l[jg],
                    "b2t": b2l[jg],
                }
            )
    return in_maps


def _gather(results):
    out = np.empty((B, I, J, K), np.complex64)
    idx = 0
    for jg in range(NJG):
        for rg in range(NRG):
            js = slice(jg * JL, (jg + 1) * JL)
            bs = slice(rg * BL, (rg + 1) * BL)
            o = np.asarray(results[idx]["out"]).astype(np.float32)  # [13,128,1024]
            oc = (o[:, :, :ROWS] + 1j * o[:, :, ROWS:]).astype(np.complex64)
            # [j, k, rows] -> [rows, j, k] -> [BL, I, JL, K]
            out[bs, :, js, :] = oc.transpose(2, 0, 1).reshape(BL, I, JL, K)
            idx += 1
    return out


def run(trace=False, **inputs):
    from concourse.bass_utils import run_bass_kernel_spmd

    if "nc" not in _cache:
        _cache["nc"] = _build_nc()
    in_maps = _prep_shards(
        np.asarray(inputs["x_real"], np.float32),
        np.asarray(inputs["x_imag"], np.float32),
        np.asarray(inputs["w1"], np.float32),
        np.asarray(inputs["b1"], np.float32),
        np.asarray(inputs["w2"], np.float32),
        np.asarray(inputs["b2"], np.float32),
    )
    res = run_bass_kernel_spmd(_cache["nc"], in_maps, list(range(8)), trace=trace)
    return _gather(res.results), res


def kernel(**inputs):
    out, _ = run(trace=False, **inputs)
    return out


# revision 40
# speedup vs baseline: 1.0239x; 1.0120x over previous
"""Trainium2 Bass kernel for nn_MlpMixer_18966575579742 (bf16 rewrite).

Complex-valued per-frequency (j) MLP:
  o1r = gelu(xr@w1[0] - xi@w1[1] + b1[0]);  o1i = gelu(xi@w1[0] + xr@w1[1] + b1[1])
  o2r = o1r@w2[0] - o1i@w2[1] + b2[0];      o2i = o1i@w2[0] + o1i@w2[1] + b2[1]
  (note: o2i intentionally uses o1i with BOTH w2[0] and w2[1], as in the source)

Sharding over 8 cores: 2 j-halves (13 each) x 4 batch-quarters (B=32 -> 512 rows).

Key differences vs the fp32 baseline (289us):
  - ALL matmul operands are bf16 (fp32 matmul = 2 HW passes + 2x DMA bytes;
    tolerance is 2e-2 absmax so bf16's ~5e-3 is plenty). 312 passes/core.
  - L1 is the DIRECT 4-matmul complex product accumulated in PSUM
    (pre_r = w1[0]^T@xr + (-w1[1])^T@xi etc.), so the Gauss-trick's 156 DVE
    combine ops are gone entirely; GELU+bias reads PSUM directly on ScalarE.
  - L2 uses the algebraic identity o2r + o2i_pre = (o1r+o1i)@w2[0] [since
    o2i_pre = o1i@(w2[0]+w2[1])]: only 2 matmuls per h-chunk:
      T = o1i@(w2[0]+w2[1])   (= o2i pre-bias)
      S = (o1r+o1i)@w2[0]     (o2r = S - T + b2r)
    costing one bf16 DVE add per h-chunk (o1s = o1r + o1i).
  - per-j DMA is 3 big contiguous transfers (x-pair 2KB/part, weight-pack
    5KB/part incl. host-negated -w1[1] and host-summed w2[0]+w2[1], out-pair
    2KB/part); weight DMAs issue on gpsimd, x/out on sync.
  - L2 matmuls are emitted with a 2-slot lag behind L1 (pending deque) so the
    PE never waits on the ScalarE GELU -> DVE add chain.
  - PSUM: 4 banks rotate L1 pre-tiles (2 h-chunks in flight), 4 banks rotate
    L2 T/S accumulators (2 j in flight). Exactly 8.
  - a dummy GELU at kernel start pulls the ~2.7us ACT table load under the
    initial DMA wait.
"""

import sys

if "/opt/trn_rl_repo" not in sys.path:
    sys.path.insert(0, "/opt/trn_rl_repo")

from collections import deque

import numpy as np
from ml_dtypes import bfloat16

B, I, J, K, F = 128, 16, 26, 128, 4
H = K * F  # 512
NJG = 2  # j groups
NRG = 4  # row (batch) groups
JL = J // NJG  # 13 j per core
BL = B // NRG  # 32 batches per core
ROWS = BL * I  # 512 rows per core
NHC = H // 128  # 4 h-chunks
WCOLS = 4 * H  # w1[0] | w1[1] | w2[0] | w2[0]+w2[1]  (-w1[1] negated on DVE)

_cache = {}


def _build_nc():
    from contextlib import ExitStack

    import concourse.mybir as mybir
    import concourse.tile as tile
    from concourse import bacc

    f32 = mybir.dt.float32
    bf16 = mybir.dt.bfloat16
    nc = bacc.Bacc(None)

    # x pre-transposed on host: [j, k, rows*2] = [xr | xi]
    xp = nc.declare_dram_parameter("xp", [JL, K, 2 * ROWS], bf16, isOutput=False)
    # weight pack: [j, 128, 5*H]; first 3 slots partition=k, last 2 partition=h%128
    wp = nc.declare_dram_parameter("wp", [JL, 128, WCOLS], bf16, isOutput=False)
    # biases host-pre-transposed to per-partition layout
    b1d = nc.declare_dram_parameter("b1t", [128, 2 * JL * NHC], f32, isOutput=False)
    b2d = nc.declare_dram_parameter("b2t", [128, 2 * JL], f32, isOutput=False)
    # transposed output: [j, k', rows*2] = [real | imag]; host fixes layout
    out = nc.declare_dram_parameter("out", [JL, K, 2 * ROWS], bf16, isOutput=True)

    GELU = mybir.ActivationFunctionType.Gelu

    with tile.TileContext(nc) as tc, ExitStack() as ctx:
        const = ctx.enter_context(tc.tile_pool(name="const", bufs=1))
        xpool = ctx.enter_context(tc.tile_pool(name="xpool", bufs=4))
        wpool = ctx.enter_context(tc.tile_pool(name="wpool", bufs=4))
        w1np = ctx.enter_context(tc.tile_pool(name="w1np", bufs=2))
        o1p = ctx.enter_context(tc.tile_pool(name="o1p", bufs=2))
        srpp = ctx.enter_context(tc.tile_pool(name="srpp", bufs=2))
        outp = ctx.enter_context(tc.tile_pool(name="outp", bufs=3))
        ps1 = ctx.enter_context(tc.tile_pool(name="ps1", bufs=4, space="PSUM"))
        ps2 = ctx.enter_context(tc.tile_pool(name="ps2", bufs=4, space="PSUM"))

        # warm tile for HAM warm-up matmuls + gelu table preload; memset on
        # gpsimd whose preamble finishes earliest, so warm-up starts ASAP
        cwarm = const.tile([128, ROWS], bf16)
        nc.gpsimd.memset(cwarm, 0)
        warm = const.tile([128, 1], bf16)
        nc.scalar.activation(warm, cwarm[:, 0:1], GELU)

        # biases arrive already transposed; straight DMA on the idle gpsimd
        # queue, issued first so they land before the first GELU
        b1t = const.tile([128, 2, JL, NHC], f32)
        nc.gpsimd.dma_start(out=b1t.rearrange("p c j hc -> p (c j hc)"), in_=b1d[:, :])
        b2t = const.tile([128, 2, JL], f32)
        nc.gpsimd.dma_start(out=b2t.rearrange("p c j -> p (c j)"), in_=b2d[:, :])

        jstate = {}

        def start_j(j, sliced=False):
            wt = wpool.tile([128, WCOLS], bf16, tag="wt")
            xt = xpool.tile([128, 2 * ROWS], bf16, tag="xt")
            if sliced:
                # all 8 cores hammer HBM simultaneously at kernel start, so a
                # monolithic first transfer gates the first matmul ~10us out.
                # Slice j0's data in compute-consumption order instead.
                nc.sync.dma_start(out=wt[:, 0:H], in_=wp[j, :, 0:H])  # w1[0]
                nc.sync.dma_start(out=xt[:, 0:ROWS], in_=xp[j, :, 0:ROWS])  # xr
                nc.sync.dma_start(out=wt[:, H : 2 * H], in_=wp[j, :, H : 2 * H])
                nc.sync.dma_start(out=xt[:, ROWS:], in_=xp[j, :, ROWS:])  # xi
                nc.sync.dma_start(out=wt[:, 2 * H :], in_=wp[j, :, 2 * H :])  # w2
            else:
                nc.sync.dma_start(out=wt, in_=wp[j])
                nc.sync.dma_start(out=xt, in_=xp[j])
            jstate[j] = (xt, wt)

        def negate_w1(j):
            # w1n = -w1[1] for the pre_r accumulation (bf16 DVE, ~0.2us)
            xt, wt = jstate[j]
            w1n = w1np.tile([128, H], bf16, tag="w1n")
            nc.vector.tensor_scalar_mul(w1n, wt[:, H : 2 * H], -1.0)
            jstate[j] = (xt, wt, w1n)

        start_j(0, sliced=True)
        start_j(1)
        negate_w1(0)

        # dummy matmuls while the first DMAs land: flips the PE HAM clock
        # gate to 8/8 so the first real passes run at 2.4 GHz
        NWARM = 9
        warmps = ps2.tile([128, ROWS], f32, tag="ps2")
        for wi in range(NWARM):
            nc.tensor.matmul(
                warmps, cwarm[:, 0:128], cwarm,
                start=(wi == 0), stop=(wi == NWARM - 1), skip_group_check=True,
            )

        TS = {}  # j -> (T, S) psum accumulators, allocated at first L2 pop

        def emit_L2(j, hc, wt, o1i, o1s, o1r=None):
            if hc == 0:
                T = ps2.tile([128, ROWS], f32, tag="ps2")
                S = ps2.tile([128, ROWS], f32, tag="ps2")
                TS[j] = (T, S)
            T, S = TS[j]
            c0 = 3 * H + hc * 128  # w2sum slot
            nc.tensor.matmul(
                T, wt[:, c0 : c0 + 128], o1i[:, hc],
                start=(hc == 0), stop=(hc == NHC - 1),
            )
            c1 = 2 * H + hc * 128  # w2[0] slot
            if o1r is not None:
                # tail shortcut (last j, last hc): feed S from o1r and o1i
                # directly so the drain needn't wait for the DVE o1s add
                nc.tensor.matmul(
                    S, wt[:, c1 : c1 + 128], o1r[:, hc], start=False, stop=False
                )
                nc.tensor.matmul(
                    S, wt[:, c1 : c1 + 128], o1i[:, hc], start=False, stop=True
                )
            else:
                nc.tensor.matmul(
                    S, wt[:, c1 : c1 + 128], o1s[:, hc],
                    start=(hc == 0), stop=(hc == NHC - 1),
                )

        def emit_drain(j):
            T, S = TS.pop(j)
            ot = outp.tile([128, 2 * ROWS], bf16, tag="ot")
            # imag: T + b2i
            nc.vector.tensor_scalar_add(ot[:, ROWS:], T, b2t[:, 1, j : j + 1])
            last = j == JL - 1
            if last:
                # ship the imag half immediately so the store overlaps the
                # remaining two DVE ops on the critical tail
                nc.sync.dma_start(out=out[j, :, ROWS:], in_=ot[:, ROWS:])
            # real: (S + b2r) - T
            srp = srpp.tile([128, ROWS], f32, tag="srp")
            nc.vector.tensor_scalar_add(srp, S, b2t[:, 0, j : j + 1])
            nc.vector.tensor_sub(ot[:, 0:ROWS], srp, T)
            if last:
                nc.sync.dma_start(out=out[j, :, 0:ROWS], in_=ot[:, 0:ROWS])
            else:
                nc.sync.dma_start(out=out[j], in_=ot)

        pend = deque()
        for j in range(JL):
            # staged prefetch ramp: depth 2 during the startup HBM crunch
            # (all 8 cores pull at once), depth 3 in steady state
            if j == 0:
                start_j(2)
            elif j + 3 < JL:
                start_j(j + 3)
            xt, wt, w1n = jstate.pop(j)
            xr_ = xt[:, 0:ROWS]
            xi_ = xt[:, ROWS:]
            o1r = o1p.tile([128, NHC, ROWS], bf16, tag="o1r")
            o1i = o1p.tile([128, NHC, ROWS], bf16, tag="o1i")
            o1s = o1p.tile([128, NHC, ROWS], bf16, tag="o1s")
            for hc in range(NHC):
                hb = hc * 128
                pr = ps1.tile([128, ROWS], f32, tag="ps1")
                pi = ps1.tile([128, ROWS], f32, tag="ps1")
                # pre_r = w1[0]^T @ xr + (-w1[1])^T @ xi   (finished first so
                # the o1r GELU can start while the pre_i passes still run)
                # pre_i = w1[0]^T @ xi +   w1[1]^T @ xr
                nc.tensor.matmul(
                    pr, wt[:, hb : hb + 128], xr_, start=True, stop=False
                )
                nc.tensor.matmul(
                    pr, w1n[:, hb : hb + 128], xi_, start=False, stop=True
                )
                # the lagged L2 passes of older h-chunks sit between pre_r
                # and pre_i, so pre_r closes early relative to the GELU that
                # consumes it (drop the lag on the last j to shorten the tail)
                while len(pend) > (1 if j < JL - 1 else 0):
                    pj, phc, pwt, po1i, po1s, po1r = pend.popleft()
                    emit_L2(pj, phc, pwt, po1i, po1s, po1r)
                    if phc == NHC - 1:
                        emit_drain(pj)
                nc.tensor.matmul(
                    pi, wt[:, hb : hb + 128], xi_, start=True, stop=False
                )
                nc.tensor.matmul(
                    pi, wt[:, H + hb : H + hb + 128], xr_, start=False, stop=True
                )
                nc.scalar.activation(
                    o1r[:, hc], pr, GELU, bias=b1t[:, 0, j, hc : hc + 1]
                )
                nc.scalar.activation(
                    o1i[:, hc], pi, GELU, bias=b1t[:, 1, j, hc : hc + 1]
                )
                tail = j == JL - 1 and hc == NHC - 1
                if not tail:
                    nc.vector.tensor_add(o1s[:, hc], o1r[:, hc], o1i[:, hc])
                pend.append((j, hc, wt, o1i, o1s, o1r if tail else None))
            if j == 0:
                start_j(3)
            if j + 1 < JL:
                # -w1[1] for the next j; its wt DMA landed during this body,
                # so this never blocks the DVE queue head
                negate_w1(j + 1)
        while pend:
            pj, phc, pwt, po1i, po1s, po1r = pend.popleft()
            emit_L2(pj, phc, pwt, po1i, po1s, po1r)
            if phc == NHC - 1:
                emit_drain(pj)

    if not nc.is_finalized():
        nc.finalize()
    return nc


def _prep_shards(x_real, x_imag, w1, b1, w2, b2):
    """Host-side packing. Returns one input map per core (8 = 2 jg x 4 rg)."""
    wpks, b1l, b2l = [], [], []
    for jg in range(NJG):
        js = slice(jg * JL, (jg + 1) * JL)
        w10 = w1[0, js]  # [JL, K, H] partition=k
        w11 = w1[1, js]
        w2z = w2[0, js]  # [JL, H, K]
        w2sum = w2[0, js] + w2[1, js]
        # [JL, H, K] -> [JL, 128, NHC*K] with partition = h % 128
        w2z_r = (
            w2z.reshape(JL, NHC, 128, K).transpose(0, 2, 1, 3).reshape(JL, 128, NHC * K)
        )
        w2s_r = (
            w2sum.reshape(JL, NHC, 128, K)
            .transpose(0, 2, 1, 3)
            .reshape(JL, 128, NHC * K)
        )
        wpk = np.concatenate([w10, w11, w2z_r, w2s_r], axis=2).astype(bfloat16)
        wpks.append(np.ascontiguousarray(wpk))
        # pre-transpose biases to the on-chip per-partition layout
        b1t = (
            b1[:, js]
            .reshape(2, JL, NHC, 128)
            .transpose(3, 0, 1, 2)
            .reshape(128, 2 * JL * NHC)
        )
        b2t = b2[:, js].transpose(2, 0, 1).reshape(128, 2 * JL)
        b1l.append(np.ascontiguousarray(b1t))
        b2l.append(np.ascontiguousarray(b2t))

    in_maps = []
    for jg in range(NJG):
        js = slice(jg * JL, (jg + 1) * JL)
        for rg in range(NRG):
            bs = slice(rg * BL, (rg + 1) * BL)
            # [BL, I, JL, K] -> [JL, K, BL*I]
            xr_s = x_real[bs, :, js, :].transpose(2, 3, 0, 1).reshape(JL, K, ROWS)
            xi_s = x_imag[bs, :, js, :].transpose(2, 3, 0, 1).reshape(JL, K, ROWS)
            xpk = np.concatenate([xr_s, xi_s], axis=2).astype(bfloat16)
            in_maps.append(
                {
                    "xp": np.ascontiguousarray(xpk),
                    "wp": wpks[jg],
                    "b1t": b1l[jg],
                    "b2t": b2l[jg],
                }
            )
    return in_maps


def _gather(results):
    out = np.empty((B, I, J, K), np.complex64)
    idx = 0
    for jg in range(NJG):
        for rg in range(NRG):
            js = slice(jg * JL, (jg + 1) * JL)
            bs = slice(rg * BL, (rg + 1) * BL)
            o = np.asarray(results[idx]["out"]).astype(np.float32)  # [13,128,1024]
            oc = (o[:, :, :ROWS] + 1j * o[:, :, ROWS:]).astype(np.complex64)
            # [j, k, rows] -> [rows, j, k] -> [BL, I, JL, K]
            out[bs, :, js, :] = oc.transpose(2, 0, 1).reshape(BL, I, JL, K)
            idx += 1
    return out


def run(trace=False, **inputs):
    from concourse.bass_utils import run_bass_kernel_spmd

    if "nc" not in _cache:
        _cache["nc"] = _build_nc()
    in_maps = _prep_shards(
        np.asarray(inputs["x_real"], np.float32),
        np.asarray(inputs["x_imag"], np.float32),
        np.asarray(inputs["w1"], np.float32),
        np.asarray(inputs["b1"], np.float32),
        np.asarray(inputs["w2"], np.float32),
        np.asarray(inputs["b2"], np.float32),
    )
    res = run_bass_kernel_spmd(_cache["nc"], in_maps, list(range(8)), trace=trace)
    return _gather(res.results), res


def kernel(**inputs):
    out, _ = run(trace=False, **inputs)
    return out
